# revision 58
# baseline (speedup 1.0000x reference)
"""MemoAttention Trainium2 kernel — 8-core SPMD.

Reference computation (B=2, S=2048, D=1024, H=16, Dh=64):
    qp = q @ Wq; kp = k @ Wk; vp = v @ Wv          (per batch)
    scores = (qh @ kh^T) * (1/8) * 2*sigmoid(qh . sw_h)   per head
    attn   = softmax(scores); out = attn @ vh
    gate   = sigmoid(out @ Wg + gb); y = (out * gate) @ Wo

Sharding: core c owns heads {2c, 2c+1} for BOTH batches (head-parallel
attention in a transposed [feature, seq] layout), then an 8-way AllToAll
converts to sequence-parallel (core c -> batch c//4, seq slice c%4) for the
gate/out_proj stage.

All device compute in bf16 (fp32 PSUM accumulate).  Host does only layout
prep (transpose / slice / dtype cast) and the final unshard.
"""

import os
import sys

import numpy as np

sys.path.insert(0, "/opt/trn_rl_repo")

import ml_dtypes

import concourse.bacc as bacc
import concourse.bass as bass
import concourse.bass_utils as bass_utils
import concourse.tile as tile
from concourse import mybir

BF16 = ml_dtypes.bfloat16

D_MODEL = 1024
NHEAD = 16
HEAD_DIM = 64
B = 2
S = 2048
N_CORES = 8
GCOLS = 2 * HEAD_DIM        # 128 d_model columns per core (2 heads)
S_CHUNK = 512
N_SC = S // S_CHUNK         # 4
N_TT = S // 128             # 16 t-tiles
N_IC = D_MODEL // 128       # 8 contraction chunks
S_SLICE = 512               # rows per core in stage 2

FP32 = mybir.dt.float32
BF16_T = mybir.dt.bfloat16

TRACE = False
TRACE_KWARGS = {}
LAST_RESULTS = None
DEBUG = False

_CACHE = {}


def _ensure_ntff_hook():
    """The agent image's antenv lacks axon_hooks; synthesize it so
    run_bass_kernel_spmd(trace=True) can NTFF-profile via libaxon_pjrt."""
    import types

    try:
        from antenv import axon_hooks  # noqa: F401
        return
    except ImportError:
        pass
    import antenv

    mod = types.ModuleType("antenv.axon_hooks")
    _state = {"hook": None}
    mod.set_axon_ntff_profile_hook = lambda h: _state.__setitem__("hook", h)
    mod.get_axon_ntff_profile_hook = lambda: _state["hook"]
    sys.modules["antenv.axon_hooks"] = mod
    antenv.axon_hooks = mod
    try:
        from trn_agent_boot.trn_boot import _ntff_profile_via_ctypes

        hook = _ntff_profile_via_ctypes("/opt/axon/libaxon_pjrt.so")
        if hook is not None:
            mod.set_axon_ntff_profile_hook(hook)
    except Exception as e:  # pragma: no cover
        print(f"ntff hook setup failed ({e}); tracing disabled", file=sys.stderr)


def _build_nc():
    nc = bacc.Bacc(
        "TRN2",
        target_bir_lowering=False,
        debug=False,
        enable_asserts=True,
        num_devices=N_CORES,
    )

    # ---- I/O ----
    qT = nc.dram_tensor("qT", [B, D_MODEL, S], BF16_T, kind="ExternalInput").ap()
    kT = nc.dram_tensor("kT", [B, D_MODEL, S], BF16_T, kind="ExternalInput").ap()
    vT = nc.dram_tensor("vT", [B, D_MODEL, S], BF16_T, kind="ExternalInput").ap()
    wq = nc.dram_tensor("wq", [128, N_IC * GCOLS], BF16_T, kind="ExternalInput").ap()
    wk = nc.dram_tensor("wk", [128, N_IC * GCOLS], BF16_T, kind="ExternalInput").ap()
    wv = nc.dram_tensor("wv", [128, N_IC * GCOLS], BF16_T, kind="ExternalInput").ap()
    swb = nc.dram_tensor("swb", [GCOLS, 2], BF16_T, kind="ExternalInput").ap()
    wg = nc.dram_tensor("wg", [128, N_IC * D_MODEL], BF16_T, kind="ExternalInput").ap()
    wo = nc.dram_tensor("wo", [128, N_IC * D_MODEL], BF16_T, kind="ExternalInput").ap()
    gb = nc.dram_tensor("gb", [128, 8], FP32, kind="ExternalInput").ap()
    yT = nc.dram_tensor("yT", [D_MODEL, S_SLICE], BF16_T, kind="ExternalOutput").ap()
    dbg = {}
    if DEBUG:
        for nm, shp, dt in [
            ("dbg_qhTs", [B, 128, S], BF16_T),
            ("dbg_kpTs", [B, 128, S], BF16_T),
            ("dbg_vp", [B, N_TT, 128, GCOLS], BF16_T),
            ("dbg_rec", [N_SC, 128, S_CHUNK], FP32),
            ("dbg_bc", [B, N_SC, 128, S_CHUNK], BF16_T),
            ("dbg_nrm", [N_CORES, 128, S_CHUNK], BF16_T),
            ("dbg_oT", [8, 128, S_SLICE], BF16_T),
            ("dbg_gt", [8, 128, S_SLICE], BF16_T),
        ]:
            dbg[nm] = nc.dram_tensor(nm, shp, dt, kind="ExternalOutput").ap()

    EXP = mybir.ActivationFunctionType.Exp
    SIG = mybir.ActivationFunctionType.Sigmoid

    with tile.TileContext(nc) as tc:
        # ---------- resident constants / weights ----------
        with tc.tile_pool(name="const", bufs=1) as cpool:
            ones_sb = cpool.tile([128, 1], BF16_T)
            nc.gpsimd.memset(ones_sb[:], 1.0)
            # stage-2 weight tiles (DMAs emitted later, after the b0 input
            # stream, so they don't compete with the critical ramp)
            wg_sb = cpool.tile([128, N_IC, D_MODEL], BF16_T)
            wo_sb = cpool.tile([128, N_IC, D_MODEL], BF16_T)
            wq_sb = cpool.tile([128, N_IC, GCOLS], BF16_T)
            nc.sync.dma_start(wq_sb[:], wq.rearrange("p (c n) -> p c n", c=N_IC))
            wk_sb = cpool.tile([128, N_IC, GCOLS], BF16_T)
            nc.sync.dma_start(wk_sb[:], wk.rearrange("p (c n) -> p c n", c=N_IC))
            wv_sb = cpool.tile([128, N_IC, GCOLS], BF16_T)
            nc.sync.dma_start(wv_sb[:], wv.rearrange("p (c n) -> p c n", c=N_IC))
            swb_sb = cpool.tile([128, 2], BF16_T)
            nc.sync.dma_start(swb_sb[:], swb)
            gb_sb = cpool.tile([128, 8], FP32)
            nc.sync.dma_start(gb_sb[:], gb)

            # persistent per-batch activation tensors
            qhTs = [cpool.tile([128, S], BF16_T, name=f"qhTs{b}") for b in range(B)]
            kpTs = [cpool.tile([128, S], BF16_T, name=f"kpTs{b}") for b in range(B)]
            vp_s = [[cpool.tile([128, GCOLS], BF16_T, name=f"vp{b}_{t}")
                     for t in range(N_TT)] for b in range(B)]
            qpT_raw = [cpool.tile([128, S], BF16_T, name=f"qpT_raw{b}")
                       for b in range(B)]

            # ---------- unified pools: proj + attention interleaved ----------
            xpool_cm = tc.tile_pool(name="xT", bufs=8)
            xpool = xpool_cm.__enter__()
            dram_cm = tc.tile_pool(name="dram", bufs=1, space="DRAM")
            dram = dram_cm.__enter__()
            a2a_in = [dram.tile([N_CORES, 128, 256], BF16_T, name=f"a2a_in{h}")
                      for h in range(2)]
            a2a_out = [dram.tile([N_CORES, 128, 256], BF16_T, name=f"a2a_out{h}")
                      for h in range(2)]
            ups_cm = tc.tile_pool(name="ups", bufs=4, space="PSUM")
            ups = ups_cm.__enter__()
            pdram_cm = tc.tile_pool(name="pdram", bufs=4, space="DRAM")
            pdram = pdram_cm.__enter__()
            scr_cm = tc.tile_pool(name="scr", bufs=3)
            scr = scr_cm.__enter__()
            attnp_cm = tc.tile_pool(name="attnp", bufs=22)
            attnp = attnp_cm.__enter__()
            nrm_cm = tc.tile_pool(name="nrm", bufs=2)
            nrm = nrm_cm.__enter__()

            def emit_proj_q(b):
                qch = []
                for i in range(N_IC):
                    t = xpool.tile([128, S], BF16_T, tag="x", name=f"qch{b}_{i}")
                    nc.sync.dma_start(t[:], qT[b, 128 * i:128 * (i + 1), :])
                    qch.append(t)
                for sc in range(N_SC):
                    ssl = slice(S_CHUNK * sc, S_CHUNK * (sc + 1))
                    ps = ups.tile([128, S_CHUNK], FP32, tag="A", bufs=4,
                                  name=f"qp{b}_{sc}")
                    for i in range(N_IC):
                        nc.tensor.matmul(
                            ps[:], lhsT=wq_sb[:, i, :], rhs=qch[i][:, ssl],
                            start=(i == 0), stop=(i == N_IC - 1))
                    nc.vector.tensor_copy(qpT_raw[b][:, ssl], ps[:])
                    # scale gate c = 0.25*sigmoid(qp . swb): head hh on
                    # partition 0 / 32 of one bank
                    psc = ups.tile([128, S_CHUNK], FP32, tag="A", bufs=4,
                                   name=f"psc{b}_{sc}")
                    for hh in range(2):
                        nc.tensor.matmul(
                            psc[32 * hh:32 * hh + 1, :],
                            lhsT=swb_sb[:, hh:hh + 1],
                            rhs=qpT_raw[b][:, ssl], start=True, stop=True,
                            tile_position=(0, 32 * hh))
                    cex = scr.tile([128, S_CHUNK], FP32, tag="cex", bufs=2)
                    nc.scalar.activation(cex[0:33, :], psc[0:33, :], EXP,
                                         scale=-1.0)
                    nc.vector.tensor_scalar_add(cex[0:33, :], cex[0:33, :], 1.0)
                    crc = scr.tile([128, S_CHUNK], FP32, tag="crc", bufs=2)
                    nc.vector.reciprocal_approx_fast(crc[0:33, :], cex[0:33, :])
                    csb = scr.tile([128, S_CHUNK], BF16_T, tag="csb")
                    nc.vector.tensor_copy(csb[0:33, :], crc[0:33, :])
                    # partition-broadcast via DRAM roundtrip (0-stride DMA)
                    cbuf = pdram.tile([1, 2 * S_CHUNK], BF16_T, tag="cbuf")
                    for hh in range(2):
                        nc.sync.dma_start(
                            cbuf[0:1, S_CHUNK * hh:S_CHUNK * (hh + 1)],
                            csb[32 * hh:32 * hh + 1, :])
                    bc = scr.tile([128, S_CHUNK], BF16_T, tag="bc")
                    for hh in range(2):
                        nc.sync.dma_start(
                            bc[64 * hh:64 * (hh + 1), :],
                            cbuf[0:1, S_CHUNK * hh:S_CHUNK * (hh + 1)]
                            .partition_broadcast(64))
                    # fold the 0.25 = (1/8 softmax) * (2 gate) factor here
                    nc.vector.scalar_tensor_tensor(
                        qhTs[b][:, ssl], qpT_raw[b][:, ssl], 0.25, bc[:],
                        op0=mybir.AluOpType.mult, op1=mybir.AluOpType.mult)
                    if DEBUG:
                        nc.vector.tensor_scalar_mul(bc[:], bc[:], 0.25)
                        nc.sync.dma_start(dbg["dbg_bc"][b, sc], bc[:])

            def emit_proj_k(b):
                kch = []
                for i in range(N_IC):
                    t = xpool.tile([128, S], BF16_T, tag="x", name=f"kch{b}_{i}")
                    nc.sync.dma_start(t[:], kT[b, 128 * i:128 * (i + 1), :])
                    kch.append(t)
                for sc in range(N_SC):
                    ssl = slice(S_CHUNK * sc, S_CHUNK * (sc + 1))
                    ps = ups.tile([128, S_CHUNK], FP32, tag="A", bufs=4,
                                  name=f"kp{b}_{sc}")
                    for i in range(N_IC):
                        nc.tensor.matmul(
                            ps[:], lhsT=wk_sb[:, i, :], rhs=kch[i][:, ssl],
                            start=(i == 0), stop=(i == N_IC - 1))
                    nc.vector.tensor_copy(kpTs[b][:, ssl], ps[:])

            def emit_proj_v(b):
                vch = []
                for i in range(N_IC):
                    t = xpool.tile([128, S], BF16_T, tag="x", name=f"vch{b}_{i}")
                    nc.sync.dma_start(t[:], vT[b, 128 * i:128 * (i + 1), :])
                    vch.append(t)
                for tt in range(N_TT):
                    ps = ups.tile([128, GCOLS], FP32, tag="A", bufs=4,
                                  name=f"vp{b}_{tt}")
                    for i in range(N_IC):
                        nc.tensor.matmul(
                            ps[:], lhsT=vch[i][:, 128 * tt:128 * (tt + 1)],
                            rhs=wv_sb[:, i, :],
                            start=(i == 0), stop=(i == N_IC - 1))
                    nc.vector.tensor_copy(vp_s[b][tt][:], ps[:])
                if DEBUG:
                    nc.sync.dma_start(dbg["dbg_qhTs"][b], qhTs[b][:])
                    nc.sync.dma_start(dbg["dbg_kpTs"][b], kpTs[b][:])
                    for tt in range(N_TT):
                        nc.sync.dma_start(dbg["dbg_vp"][b, tt], vp_s[b][tt][:])

            def emit_attn_sc(b, sc):
                ssl = slice(S_CHUNK * sc, S_CHUNK * (sc + 1))
                den = ups.tile([128, S_CHUNK], FP32, tag="A", bufs=4,
                               name=f"den{b}_{sc}")
                av = ups.tile([128, S_CHUNK], FP32, tag="A", bufs=4,
                              name=f"av{b}_{sc}")
                ats = []
                for tt in range(N_TT):
                    tsl = slice(128 * tt, 128 * (tt + 1))
                    sps = ups.tile([128, 2 * S_CHUNK], FP32, tag="sc", bufs=2,
                                   name=f"sc{b}_{sc}_{tt}")
                    for hh in range(2):  # row-tiled K=64 pair
                        rows = slice(64 * hh, 64 * (hh + 1))
                        nc.tensor.matmul(
                            sps[:, S_CHUNK * hh:S_CHUNK * (hh + 1)],
                            lhsT=kpTs[b][rows, tsl],
                            rhs=qhTs[b][rows, ssl],
                            start=True, stop=True)
                    at = attnp.tile([128, 2 * S_CHUNK], BF16_T, tag="at",
                                    name=f"at{b}_{sc}_{tt}", bufs=22)
                    nc.scalar.activation(at[:], sps[:], EXP)
                    for hh in range(2):  # attn @ V, col-tiled pair
                        nc.tensor.matmul(
                            av[64 * hh:64 * (hh + 1), :],
                            lhsT=vp_s[b][tt][:, 64 * hh:64 * (hh + 1)],
                            rhs=at[:, S_CHUNK * hh:S_CHUNK * (hh + 1)],
                            start=(tt == 0), stop=(tt == N_TT - 1),
                            skip_group_check=True)
                    ats.append(at)
                    if tt % 2 == 1:  # pairwise tree level 1 (DVE)
                        nc.vector.tensor_add(
                            ats[tt - 1][:], ats[tt - 1][:], ats[tt][:])
                stride = 2
                while stride < N_TT:
                    for i in range(0, N_TT, 2 * stride):
                        nc.vector.tensor_add(
                            ats[i][:], ats[i][:], ats[i + stride][:])
                    stride *= 2
                # denominator = ones^T @ (summed exp), col-tiled pair
                for hh in range(2):
                    nc.tensor.matmul(
                        den[32 * hh:32 * hh + 1, :],
                        lhsT=ones_sb[:],
                        rhs=ats[0][:, S_CHUNK * hh:S_CHUNK * (hh + 1)],
                        start=True, stop=True,
                        tile_position=(0, 32 * hh),
                        skip_group_check=True)
                # evacuate av to SBUF immediately: frees its PSUM slot for the
                # next chunk before the latency-bound normalize chain below
                av_sb = nrm.tile([128, S_CHUNK], FP32, tag="avsb")
                nc.vector.tensor_copy(av_sb[:], av[:])
                # normalize + stage A2A input (shard d = b*4 + sc)
                rec = nrm.tile([128, S_CHUNK], FP32, tag="rec")
                nc.vector.reciprocal_approx_fast(rec[:], den[:])
                if DEBUG and b == 0:
                    nc.sync.dma_start(dbg["dbg_rec"][sc], rec[:])
                rst = pdram.tile([1, 2 * S_CHUNK], FP32, tag="rst")
                for hh in range(2):
                    nc.sync.dma_start(
                        rst[0:1, S_CHUNK * hh:S_CHUNK * (hh + 1)],
                        rec[32 * hh:32 * hh + 1, :])
                bcs = nrm.tile([128, S_CHUNK], FP32, tag="bcs")
                for hh in range(2):
                    nc.sync.dma_start(
                        bcs[64 * hh:64 * (hh + 1), :],
                        rst[0:1, S_CHUNK * hh:S_CHUNK * (hh + 1)]
                        .partition_broadcast(64))
                nrm_t = nrm.tile([128, S_CHUNK], BF16_T, tag="nrmt")
                nc.vector.tensor_mul(nrm_t[:], av_sb[:], bcs[:])
                nc.sync.dma_start(a2a_in[b][2 * sc, :, :], nrm_t[:, 0:256])
                nc.sync.dma_start(a2a_in[b][2 * sc + 1, :, :], nrm_t[:, 256:512])
                if DEBUG:
                    nc.sync.dma_start(dbg["dbg_nrm"][4 * b + sc], nrm_t[:])


            s2sb_cm = tc.tile_pool(name="s2sb", bufs=1)
            s2sb = s2sb_cm.__enter__()

            def emit_stage2(half):
                of = a2a_out[half].rearrange("a b c -> (a b) c")
                oT = []
                for k in range(8):
                    t = s2sb.tile([128, 256], BF16_T, name=f"oT{half}_{k}")
                    nc.sync.dma_start(t[:], of[128 * k:128 * (k + 1), :])
                    oT.append(t)
                gt = []
                for jc in range(N_IC):
                    gps = ups.tile([128, 256], FP32, tag="A", bufs=4,
                                   name=f"gps{half}_{jc}")
                    for j2 in range(N_IC):
                        nc.tensor.matmul(
                            gps[:], lhsT=wg_sb[:, j2, 128 * jc:128 * (jc + 1)],
                            rhs=oT[j2][:],
                            start=(j2 == 0), stop=(j2 == N_IC - 1))
                    sgx = s2sb.tile([128, 256], FP32, tag="sgx", bufs=2,
                                    name=f"sgx{half}_{jc}")
                    nc.scalar.activation(sgx[:], gps[:], EXP, scale=-1.0,
                                         bias=gb_sb[:, jc:jc + 1])
                    nc.vector.tensor_scalar_add(sgx[:], sgx[:], 1.0)
                    sgr = s2sb.tile([128, 256], FP32, tag="sgr", bufs=2,
                                    name=f"sgr{half}_{jc}")
                    nc.vector.reciprocal_approx_fast(sgr[:], sgx[:])
                    sg = s2sb.tile([128, 256], BF16_T, tag="sg", bufs=2,
                                   name=f"sg{half}_{jc}")
                    nc.vector.tensor_copy(sg[:], sgr[:])
                    g = s2sb.tile([128, 256], BF16_T, tag=f"gt{jc}", bufs=2,
                                  name=f"gt{half}_{jc}")
                    nc.vector.tensor_mul(g[:], oT[jc][:], sg[:])
                    gt.append(g)
                for ct in range(8):
                    yps = ups.tile([128, 256], FP32, tag="A", bufs=4,
                                   name=f"yps{half}_{ct}")
                    for jc in range(N_IC):
                        nc.tensor.matmul(
                            yps[:], lhsT=wo_sb[:, jc, 128 * ct:128 * (ct + 1)],
                            rhs=gt[jc][:],
                            start=(jc == 0), stop=(jc == N_IC - 1))
                    yo = s2sb.tile([128, 256], BF16_T, tag="yo", bufs=2,
                                   name=f"yo{half}_{ct}")
                    nc.vector.tensor_copy(yo[:], yps[:])
                    nc.sync.dma_start(
                        yT[128 * ct:128 * (ct + 1), 256 * half:256 * (half + 1)],
                        yo[:])

            # drive: proj(b0); then attn(b0) sc-chunks interleaved with proj(b1)
            # (k first: scores need the full kpTs, but only the sc0 slice of q)
            emit_proj_k(0)
            emit_proj_q(0)
            emit_proj_v(0)
            for sc in range(N_SC):
                emit_attn_sc(0, sc)
                if sc == 0:
                    emit_proj_q(1)
                elif sc == 1:
                    emit_proj_k(1)
                elif sc == 2:
                    emit_proj_v(1)
                else:
                    nc.sync.dma_start(
                        wg_sb[:], wg.rearrange("p (c n) -> p c n", c=N_IC))
                    nc.sync.dma_start(
                        wo_sb[:], wo.rearrange("p (c n) -> p c n", c=N_IC))
            nc.gpsimd.collective_compute(
                "AllToAll", mybir.AluOpType.bypass,
                replica_groups=[list(range(N_CORES))],
                ins=[a2a_in[0].opt()], outs=[a2a_out[0].opt()])
            for sc in range(N_SC):
                emit_attn_sc(1, sc)
            nc.gpsimd.collective_compute(
                "AllToAll", mybir.AluOpType.bypass,
                replica_groups=[list(range(N_CORES))],
                ins=[a2a_in[1].opt()], outs=[a2a_out[1].opt()])
            # stage-2 half 0 (data ready since A2A#1) fills the A2A#2 window
            emit_stage2(0)
            emit_stage2(1)

            for cm in (s2sb_cm, nrm_cm, attnp_cm, scr_cm, pdram_cm, ups_cm, xpool_cm):
                cm.__exit__(None, None, None)

            dram_cm.__exit__(None, None, None)

    nc.compile()
    return nc


def _shard_inputs(q, k, v, q_proj_weight, k_proj_weight, v_proj_weight,
                  out_proj_weight, gate_weight, gate_bias, scale_weight):
    in_maps = []
    gbh = np.ascontiguousarray(
        -gate_bias.astype(np.float32).reshape(8, 128).T)  # [128, 8], negated
    def _prearr(w):  # [1024, N] -> [128, 8*N]: row p holds chunks c at (c*N..)
        n = w.shape[1]
        return np.ascontiguousarray(
            w.reshape(8, 128, n).transpose(1, 0, 2).reshape(128, 8 * n)).astype(BF16)

    wg_h = _prearr(gate_weight)
    wo_h = _prearr(out_proj_weight)
    qT = np.ascontiguousarray(q.transpose(0, 2, 1)).astype(BF16)
    kT = np.ascontiguousarray(k.transpose(0, 2, 1)).astype(BF16)
    vT = np.ascontiguousarray(v.transpose(0, 2, 1)).astype(BF16)
    for c in range(N_CORES):
        cols = slice(GCOLS * c, GCOLS * (c + 1))
        swb = np.zeros((GCOLS, 2), np.float32)
        swb[0:64, 0] = scale_weight[2 * c]
        swb[64:128, 1] = scale_weight[2 * c + 1]
        in_maps.append({
            "qT": qT,
            "kT": kT,
            "vT": vT,
            "wq": _prearr(q_proj_weight[:, cols]),
            "wk": _prearr(k_proj_weight[:, cols]),
            "wv": _prearr(v_proj_weight[:, cols]),
            "swb": swb.astype(BF16),
            "wg": wg_h,
            "wo": wo_h,
            "gb": gbh,
        })
    return in_maps


def kernel(**inputs):
    global LAST_RESULTS
    if "nc" not in _CACHE:
        _CACHE["nc"] = _build_nc()
    nc = _CACHE["nc"]
    if TRACE:
        _ensure_ntff_hook()
    in_maps = _shard_inputs(**{k: np.asarray(v) for k, v in inputs.items()})
    res = bass_utils.run_bass_kernel_spmd(
        nc, in_maps, core_ids=list(range(N_CORES)),
        trace=TRACE, trace_kwargs=TRACE_KWARGS,
    )
    LAST_RESULTS = res
    y = np.zeros((B, S, D_MODEL), np.float32)
    for c in range(N_CORES):
        yt = np.asarray(res.results[c]["yT"], np.float32)
        y[0, 256 * c:256 * (c + 1), :] = yt[:, 0:256].T
        y[1, 256 * c:256 * (c + 1), :] = yt[:, 256:512].T
    return y


if __name__ == "__main__":
    rng = np.random.default_rng(0)
    fake = {
        "q": rng.normal(size=(B, S, D_MODEL)).astype(np.float32),
        "k": rng.normal(size=(B, S, D_MODEL)).astype(np.float32),
        "v": rng.normal(size=(B, S, D_MODEL)).astype(np.float32),
        "q_proj_weight": rng.normal(size=(D_MODEL, D_MODEL)).astype(np.float32) * 0.02,
        "k_proj_weight": rng.normal(size=(D_MODEL, D_MODEL)).astype(np.float32) * 0.02,
        "v_proj_weight": rng.normal(size=(D_MODEL, D_MODEL)).astype(np.float32) * 0.02,
        "out_proj_weight": rng.normal(size=(D_MODEL, D_MODEL)).astype(np.float32) * 0.02,
        "gate_weight": rng.normal(size=(D_MODEL, D_MODEL)).astype(np.float32) * 0.02,
        "gate_bias": rng.normal(size=(D_MODEL,)).astype(np.float32) * 0.02,
        "scale_weight": rng.normal(size=(NHEAD, HEAD_DIM)).astype(np.float32) * 0.02,
    }
    out = kernel(**fake)
    print("ran", out.shape, out.dtype)
